# revision 1
# baseline (speedup 1.0000x reference)
"""MD5Surrogate Bass kernel for 8x TRN2 NeuronCores.

Data-parallel over batch (2048 rows/core). The 64-round scan is split into
60 "cheap" rounds and 4 exact tail rounds, exploiting the strong per-round
contraction of the recurrence (a state perturbation decays ~10x per round,
so early-round errors are invisible in the final state; simulated
end-to-end error vs the fp32 reference is ~8e-4 against the 2e-2 gate).

Cheap rounds: all matmuls in fp8(e4m3); L1/L2 use DoubleRow perf mode
(K=256 contracted in one pass at 2x rate). Gelu is split across engines:
the scalar engine computes exact gelu for the j0 feature half and all of
h2; the h1/j1 feature half uses a clamp approximation with a single PSUM
read:
    u = s*z + b         (DVE tensor_scalar, PSUM -> SBUF bf16)
    v = u - b           (DVE, == s*z)
    c = clamp(u, 0, 1)  (gpsimd)
    h = v * c           (gpsimd, == s * z * clamp(s*z+b, 0, 1))
The uniform s factor is folded host-side into the W2 contraction rows that
consume h1/j1 (pre-divided by s), so no rescale op is needed. gpsimd never
touches PSUM (illegal on trn2), and scalar_tensor_tensor is not available
on gpsimd - hence the v/c split above.

The work is software-pipelined as phases of one 1024-column pair each
(2 phases per round; x/state for pair p lives on partitions 64p, psum
tiles [128,1024] on 4 rotating buffers = all 8 banks). Slot ph emits
h2+L3+epilogue pieces of phase ph-1 interleaved with L1/L2 of phase ph in
expected-readiness order, so the in-order engine queues track the actual
execution order and the two pair chains overlap. L3 packs both pairs into
one psum tile at partition offsets 0/64 (M padded to 64 with zero weight
columns, plain accumulating matmuls so the k=0 half overlaps the h2/j1
act; DoubleRow would also require dst partition base 0), making the state
epilogue two 512-wide vector ops per pair.

Tail rounds (60-63): the proven fp32r path with exact scalar gelu,
restoring full precision before the output.
"""

import sys

sys.path.insert(0, "/opt/trn_rl_repo")

import numpy as np
import ml_dtypes

NUM_ROUNDS = 64
DH = 256
B = 16384
NCORES = 8
BC = B // NCORES          # batch per core = 2048
GW = 512                  # PSUM bank width (fp32) = matmul N per instruction
GPW = 1024                # column group/pair width (2 groups, partitions 0/64)
SW = 1024                 # tail stream width
N_TAIL = 4
N_CHEAP = NUM_ROUNDS - N_TAIL
HANDOFF = N_CHEAP - 1     # last cheap round writes the tail's x tiles

# gelu clamp approximation: y = z * clamp(S_GELU*z + B_GELU, 0, 1)
S_GELU = 0.2775
B_GELU = 0.5
# number of leading 512-col chunks of h2/j1 computed via the approx path
# (the rest of h2 plus all of h1/j0 is exact on the scalar engine)
H2A_CHUNKS = 0

SLAB_F = 804              # tail (fp32r) slab layout, same as the old kernel

_SCHED = np.array(
    [i if i < 16 else ((5 * i + 1) % 16 if i < 32 else ((3 * i + 5) % 16 if i < 48 else (7 * i) % 16))
     for i in range(64)],
    dtype=np.int32,
)
_SHIFT = np.array(
    [7, 12, 17, 22] * 4 + [5, 9, 14, 20] * 4 + [4, 11, 16, 23] * 4 + [6, 10, 15, 21] * 4,
    dtype=np.float32,
)
_ROUND_INFO = np.stack(
    [np.arange(64, dtype=np.float32) / 64.0, _SHIFT / 25.0], axis=-1
)  # (64, 2)

F8NP = ml_dtypes.float8_e4m3

_COMPILED = {}


def _build(time_loop_iters=0):
    import concourse.bass as bass  # noqa: F401
    from concourse import bacc
    import concourse.mybir as mybir
    from concourse.tile import TileContext

    F32 = mybir.dt.float32
    F32R = mybir.dt.float32r
    F8 = mybir.dt.float8e4
    BF16 = mybir.dt.bfloat16
    AF = mybir.ActivationFunctionType
    DR = mybir.MatmulPerfMode.DoubleRow
    ALU = mybir.AluOpType

    nc = bacc.Bacc()
    st8_d = nc.dram_tensor("st8", [128, GPW], F8, kind="ExternalInput")
    msg8_d = nc.dram_tensor("msg8", [128, 16, GPW], F8, kind="ExternalInput")
    wsl8_d = nc.dram_tensor("wsl8", [N_CHEAP, 128, 2, 640], F8, kind="ExternalInput")
    bias8_d = nc.dram_tensor("bias8", [128, 4 * N_CHEAP + 4], F32, kind="ExternalInput")
    msgt_d = nc.dram_tensor("msgt", [65, BC], F32R, kind="ExternalInput")
    slabt_d = nc.dram_tensor("slabt", [N_TAIL, 128, SLAB_F], F32R, kind="ExternalInput")
    biast_d = nc.dram_tensor("biast", [128, 4 * N_TAIL], F32, kind="ExternalInput")
    out_d = nc.dram_tensor("out", [16, BC], F32, kind="ExternalOutput")

    B3H = 4 * N_CHEAP  # col of bias8 holding the handoff round's classic b3

    with TileContext(nc) as tc:
        with tc.tile_pool(name="cpool", bufs=1) as cpool, \
             tc.tile_pool(name="wpool", bufs=6) as wpool, \
             tc.tile_pool(name="upool", bufs=2) as upool, \
             tc.tile_pool(name="pspool", bufs=4, space="PSUM") as ps:

            x8 = cpool.tile([128, 2, GPW], F8, name="x8")
            h1 = cpool.tile([128, 2, BC], F8, name="h1")
            h2 = cpool.tile([128, 2, BC], F8, name="h2")
            b8 = cpool.tile([128, 4 * N_CHEAP + 4], F32, name="b8")
            bT = cpool.tile([128, 4 * N_TAIL], F32, name="bT")
            xT = [cpool.tile([21, SW], F32R, name=f"xT{s}") for s in range(2)]
            h1T = [cpool.tile([128, 2 * SW], F32R, name=f"h1T{s}") for s in range(2)]
            h2T = [cpool.tile([128, 2 * SW], F32R, name=f"h2T{s}") for s in range(2)]
            outt = cpool.tile([16, BC], F32, name="outt")

            # prologue DMAs
            nc.sync.dma_start(x8[:, 0, :], st8_d[:])
            nc.sync.dma_start(x8[:, 1, :], msg8_d[:, int(_SCHED[0]), :])
            nc.sync.dma_start(b8[:], bias8_d[:])
            nc.sync.dma_start(bT[:], biast_d[:])
            for s in range(2):
                nc.sync.dma_start(xT[s][16:17, :], msgt_d[64:65, s * SW:(s + 1) * SW])

            wTs = []

            # --- software-pipelined cheap phases -------------------------
            # One phase = one column pair p of one round r (ph = 2r + p).
            # Slot ph emits, in expected-readiness order:
            #   L1(ph) | TS1(ph) | L3(ph-1)+epi(ph-1) | h1 acts/ops (ph)
            #   | L2(ph) | h2 acts/ops (ph)
            # so each in-order engine queue matches the actual execution
            # order and the two pairs' chains interleave across rounds.
            PH = {}        # ph -> dict with tiles/metadata
            w8s = {}       # round -> weight slab tile

            def phase_front(ph):
                """L1 + TS1 + word prefetch for phase ph."""
                r, p = divmod(ph, 2)
                if p == 0:
                    w8 = wpool.tile([128, 2, 640], F8, tag="w8", name="w8")
                    w8s[r] = w8
                    nc.sync.dma_start(w8[:], wsl8_d[r])
                    if N_CHEAP - 9 <= r < N_CHEAP - 9 + N_TAIL:
                        t = r - (N_CHEAP - 9)
                        wT = wpool.tile([128, SLAB_F], F32R, tag="wT",
                                        bufs=N_TAIL, name="wT")
                        wTs.append(wT)
                        nc.sync.dma_start(wT[:], slabt_d[t])
                    if r >= 2:
                        del w8s[r - 2]
                w8 = w8s[r]
                st = PH[ph] = {"r": r, "p": p, "w8": w8}
                lo = p * GPW
                ps1 = [ps.tile([128, GPW], F32, tag="big", name=f"ps1_{ph}_{j}")
                       for j in range(2)]
                st["ps1"] = ps1
                u = upool.tile([128, GPW], BF16, tag="uh1", name="u1")
                v = upool.tile([128, GPW], BF16, tag="vh1", name="v1")
                cc = upool.tile([128, GPW], BF16, tag="ch1", name="cc1")
                # h1 j1 approx per chunk c:
                #   u = s*z + b [DVE psum->bf16]; v = u - b [DVE]
                #   c = clamp(u,0,1) [gpsimd]; h = v*c [gpsimd] == s*z*clamp
                for c in range(2):
                    cs = slice(c * GW, (c + 1) * GW)
                    for j in range(2):
                        nc.tensor.matmul(
                            ps1[j][:, cs],
                            w8[64 * p:64 * p + 16, :, 256 + j * 128:256 + (j + 1) * 128],
                            x8[64 * p:64 * p + 16, :, cs],
                            start=True, stop=True, perf_mode=DR)
                    nc.vector.tensor_scalar(
                        u[:, cs], ps1[1][:, cs],
                        S_GELU, B_GELU, ALU.mult, ALU.add)
                for c in range(2):
                    cs = slice(c * GW, (c + 1) * GW)
                    nc.vector.tensor_scalar(
                        v[:, cs], u[:, cs], B_GELU, None, ALU.subtract)
                    nc.vector.tensor_scalar(cc[:, cs], u[:, cs], 0.0, 1.0,
                                            ALU.max, ALU.min)
                    nc.vector.tensor_tensor(
                        h1[:, 1, lo + c * GW:lo + (c + 1) * GW],
                        v[:, cs], cc[:, cs], ALU.mult)
                if r + 1 < N_CHEAP:
                    nc.sync.dma_start(
                        x8[64 * p:64 * p + 64, 1, :],
                        msg8_d[64 * p:64 * p + 64, int(_SCHED[r + 1]), :])

            def phase_mid1(ph):
                """h1 j0 act + L2 for phase ph."""
                st = PH[ph]
                r, p, w8 = st["r"], st["p"], st["w8"]
                ps1 = st["ps1"]
                lo = p * GPW
                for c in range(2):
                    nc.scalar.activation(
                        h1[:, 0, lo + c * GW:lo + (c + 1) * GW],
                        ps1[0][:, c * GW:(c + 1) * GW], AF.Gelu)
                # L2
                ps2 = [ps.tile([128, GPW], F32, tag="big", name=f"ps2_{ph}_{j}")
                       for j in range(2)]
                st["ps2"] = ps2
                for j in range(2):
                    for c in range(2):
                        nc.tensor.matmul(
                            ps2[j][:, c * GW:(c + 1) * GW],
                            w8[:, :, j * 128:(j + 1) * 128],
                            h1[:, :, lo + c * GW:lo + (c + 1) * GW],
                            start=True, stop=True, perf_mode=DR)

            def phase_mid2(ph):
                """h2 acts/ops for phase ph (emitted one slot later)."""
                st = PH[ph]
                r, p = st["r"], st["p"]
                ps2 = st["ps2"]
                lo = p * GPW
                # h2 j0 exact
                nc.scalar.activation(
                    h2[:, 0, lo:lo + GPW], ps2[0][:], AF.Gelu,
                    bias=b8[:, 4 * r:4 * r + 1])
                # h2 j1: pair A leading chunks approx (bias enters u), rest
                # exact on scalar
                if p == 0 and H2A_CHUNKS > 0:
                    AW = H2A_CHUNKS * GW
                    u2 = upool.tile([128, AW], BF16, tag="uh2", name="u2")
                    v2 = upool.tile([128, AW], BF16, tag="vh2", name="v2")
                    c2 = upool.tile([128, AW], BF16, tag="ch2", name="c2")
                    nc.vector.tensor_scalar(
                        u2, ps2[1][:, 0:AW], S_GELU,
                        b8[:, 4 * r + 2:4 * r + 3], ALU.mult, ALU.add)
                    nc.vector.tensor_scalar(
                        v2, u2, B_GELU, None, ALU.subtract)
                    nc.vector.tensor_scalar(c2, u2, 0.0, 1.0, ALU.max, ALU.min)
                    nc.vector.tensor_tensor(h2[:, 1, 0:AW], v2, c2, ALU.mult)
                    nc.scalar.activation(
                        h2[:, 1, AW:GPW], ps2[1][:, AW:GPW], AF.Gelu,
                        bias=b8[:, 4 * r + 1:4 * r + 2])
                else:
                    nc.scalar.activation(
                        h2[:, 1, lo:lo + GPW], ps2[1][:], AF.Gelu,
                        bias=b8[:, 4 * r + 1:4 * r + 2])

            def phase_back(ph):
                """L3 + state epilogue for phase ph (pair-local)."""
                st = PH.pop(ph)
                r, p, w8, ps2 = st["r"], st["p"], st["w8"], st["ps2"]
                if r != HANDOFF:
                    # plain accumulating matmuls (not DoubleRow): the k=0
                    # half only needs h2/j0, so L3 starts while the h2/j1
                    # act is still running
                    for k in range(2):
                        for c in range(2):
                            cg = 2 * p + c
                            w3lo = 512 if cg < H2A_CHUNKS else 576
                            nc.tensor.matmul(
                                ps2[0][64 * p:64 * p + 64, c * GW:(c + 1) * GW],
                                w8[:, k, w3lo:w3lo + 64],
                                h2[:, k, cg * GW:(cg + 1) * GW],
                                start=(k == 0), stop=(k == 1))
                    # per-chunk epilogue so the next round's L1 for this
                    # chunk can start as soon as its half is written
                    for c in range(2):
                        nc.vector.tensor_scalar(
                            x8[64 * p:64 * p + 64, 0, c * GW:(c + 1) * GW],
                            ps2[0][64 * p:64 * p + 64, c * GW:(c + 1) * GW],
                            b8[64 * p:64 * p + 64, 4 * r + 3:4 * r + 4],
                            None, ALU.add)
                else:
                    # handoff: classic L3 (M=16, base 0) into the tail's x
                    for c in range(2):
                        cg = 2 * p + c
                        w3lo = 512 if cg < H2A_CHUNKS else 576
                        sl = ps2[1][0:16, c * GW:(c + 1) * GW]
                        nc.tensor.matmul(
                            sl, w8[:, :, w3lo:w3lo + 16],
                            h2[:, :, cg * GW:(cg + 1) * GW],
                            start=True, stop=True, perf_mode=DR)
                        nc.vector.tensor_scalar(
                            xT[p][0:16, c * 512:(c + 1) * 512], sl,
                            b8[0:16, B3H:B3H + 1], None, ALU.add)

            def tail_round(t, is_last):
                slab = wTs[t]
                W2v = slab[:, 0:512]
                W3v = slab[:, 512:544]
                W1v = slab[0:21, 547:803]
                g = int(_SCHED[N_CHEAP + t])
                for s in range(2):
                    nc.sync.dma_start(
                        xT[s][17:21, :], msgt_d[4 * g:4 * g + 4, s * SW:(s + 1) * SW])
                pt1 = {}
                for s in range(2):
                    for j in range(2):
                        p1 = ps.tile([128, GPW], F32, tag="big", name=f"pt1_{s}{j}")
                        pt1[s, j] = p1
                        for b in range(2):
                            nc.tensor.matmul(
                                p1[:, b * 512:(b + 1) * 512],
                                W1v[:, j * 128:(j + 1) * 128],
                                xT[s][:, b * 512:(b + 1) * 512],
                                start=True, stop=True)
                        nc.scalar.activation(
                            h1T[s][:, j * SW:(j + 1) * SW], p1[:], AF.Gelu)
                for s in range(2):
                    pt2 = {}
                    for j in range(2):
                        p2 = ps.tile([128, GPW], F32, tag="big", name=f"pt2_{s}{j}")
                        pt2[j] = p2
                        for b in range(2):
                            for k in range(2):
                                nc.tensor.matmul(
                                    p2[:, b * 512:(b + 1) * 512],
                                    W2v[:, (2 * k + j) * 128:(2 * k + j + 1) * 128],
                                    h1T[s][:, k * SW + b * 512:k * SW + (b + 1) * 512],
                                    start=(k == 0), stop=(k == 1))
                        nc.scalar.activation(
                            h2T[s][:, j * SW:(j + 1) * SW], p2[:], AF.Gelu,
                            bias=bT[:, 4 * t + j:4 * t + j + 1])
                    for b in range(2):
                        sl = pt2[b][0:16, 512:1024]
                        for k in range(2):
                            nc.tensor.matmul(
                                sl, W3v[:, k * 16:(k + 1) * 16],
                                h2T[s][:, k * SW + b * 512:k * SW + (b + 1) * 512],
                                start=(k == 0), stop=(k == 1))
                        b3v = bT[0:16, 4 * t + 2:4 * t + 3]
                        if is_last:
                            nc.vector.tensor_scalar(
                                outt[:, s * SW + b * 512:s * SW + (b + 1) * 512],
                                sl, b3v, None, ALU.add)
                        else:
                            nc.vector.tensor_scalar(
                                xT[s][0:16, b * 512:(b + 1) * 512], sl, b3v,
                                None, ALU.add)

            def whole_pass():
                NP = 2 * N_CHEAP
                for ph in range(NP):
                    if ph >= 1:
                        phase_mid2(ph - 1)
                    phase_front(ph)
                    if ph >= 1:
                        phase_back(ph - 1)
                    phase_mid1(ph)
                phase_mid2(NP - 1)
                phase_back(NP - 1)
                for t in range(N_TAIL):
                    tail_round(t, t == N_TAIL - 1)
                wTs.clear()
                w8s.clear()

            if time_loop_iters:
                with tc.For_i(0, time_loop_iters, 1):
                    whole_pass()
            else:
                whole_pass()

            nc.sync.dma_start(out_d[:], outt[:])

    nc.compile()
    return nc


def _prep_host(message_bytes, initial_state, W1, b1, W2, b2, W3, b3):
    """Build per-core input tensors. Returns a list of dicts (one per core)."""
    message_bytes = np.asarray(message_bytes, dtype=np.float32)
    initial_state = np.asarray(initial_state, dtype=np.float32)
    W1 = np.asarray(W1, dtype=np.float32)
    b1 = np.asarray(b1, dtype=np.float32)
    W2 = np.asarray(W2, dtype=np.float32)
    b2 = np.asarray(b2, dtype=np.float32)
    W3 = np.asarray(W3, dtype=np.float32)
    b3 = np.asarray(b3, dtype=np.float32)
    inv_s = 1.0 / S_GELU

    # ---- shared (weight) tensors ----
    # cheap-round fp8 slab [N_CHEAP, 128, 2, 640]:
    #   [0:256)   W2pack   (k=1 contraction rows divided by s: h1/j1 is scaled)
    #   [256:512) W1pack   (state half | const+word half), replicated at 0/64
    #   [512:576) W3pack scaled variant (k=1 rows / s), M padded to 64
    #   [576:640) W3pack plain variant, M padded to 64
    wsl8 = np.zeros((N_CHEAP, 128, 2, 640), dtype=np.float32)
    bias8 = np.zeros((128, 4 * N_CHEAP + 4), dtype=np.float32)
    for i in range(N_CHEAP):
        p = np.arange(128)
        for k in range(2):
            scale = inv_s if k == 1 else 1.0
            wsl8[i, :, k, 0:256] = W2[i][k * 128 + p, :] * scale
            wsl8[i, :, k, 512:528] = W3[i][k * 128 + p, :] * scale
            wsl8[i, :, k, 576:592] = W3[i][k * 128 + p, :]
        b1p = b1[i] + _ROUND_INFO[i] @ W1[i][20:22]
        for g in range(2):
            sl = slice(64 * g, 64 * g + 16)
            wsl8[i, sl, 0, 256:512] = W1[i][0:16]
            wsl8[i, 64 * g + 0, 1, 256:512] = b1p
            wsl8[i, 64 * g + 1:64 * g + 5, 1, 256:512] = W1[i][16:20]
            bias8[sl, 4 * i + 3] = b3[i]
        bias8[:, 4 * i + 0] = b2[i][0:128]
        bias8[:, 4 * i + 1] = b2[i][128:256]
        bias8[:, 4 * i + 2] = S_GELU * b2[i][128:256] + B_GELU
    bias8[0:16, 4 * N_CHEAP] = b3[HANDOFF]
    wsl8 = wsl8.astype(F8NP)

    # tail fp32 slab (same layout as the old all-fp32r kernel)
    slabt = np.zeros((N_TAIL, 128, SLAB_F), dtype=np.float32)
    biast = np.zeros((128, 4 * N_TAIL), dtype=np.float32)
    for t in range(N_TAIL):
        i = N_CHEAP + t
        W2i = W2[i]
        slabt[t, :, 0:128] = W2i[0:128, 0:128]
        slabt[t, :, 128:256] = W2i[0:128, 128:256]
        slabt[t, :, 256:384] = W2i[128:256, 0:128]
        slabt[t, :, 384:512] = W2i[128:256, 128:256]
        W3i = W3[i]
        slabt[t, :, 512:528] = W3i[0:128, :]
        slabt[t, :, 528:544] = W3i[128:256, :]
        biast[:, 4 * t + 0] = b2[i][0:128]
        biast[:, 4 * t + 1] = b2[i][128:256]
        biast[0:16, 4 * t + 2] = b3[i]
        b1p = b1[i] + _ROUND_INFO[i] @ W1[i][20:22]
        slabt[t, 0:16, 547:803] = W1[i][0:16]
        slabt[t, 16, 547:803] = b1p
        slabt[t, 17:21, 547:803] = W1[i][16:20]

    # ---- per-core (batch-sharded) tensors ----
    in_maps = []
    for c in range(NCORES):
        cols = slice(c * BC, (c + 1) * BC)
        mb = message_bytes[cols]          # (BC, 64)
        st = initial_state[cols]          # (BC, 16)
        st8 = np.zeros((128, GPW), dtype=np.float32)
        msg8 = np.zeros((128, 16, GPW), dtype=np.float32)
        for g in range(2):
            gc = slice(g * GPW, (g + 1) * GPW)
            st8[64 * g:64 * g + 16, :] = st[gc].T
            msg8[64 * g + 0, :, :] = 1.0
            for k in range(4):
                msg8[64 * g + 1 + k, :, :] = mb[gc, k::4].T  # word w, byte k
        msgt = np.empty((65, BC), dtype=np.float32)
        msgt[0:64] = mb.T
        msgt[64] = 1.0
        in_maps.append({
            "st8": st8.astype(F8NP),
            "msg8": msg8.astype(F8NP),
            "wsl8": wsl8,
            "bias8": bias8,
            "msgt": msgt,
            "slabt": slabt,
            "biast": biast,
        })
    return in_maps


def kernel(message_bytes, initial_state, W1, b1, W2, b2, W3, b3):
    from concourse.bass_utils import run_bass_kernel_spmd

    if "nc" not in _COMPILED:
        _COMPILED["nc"] = _build()
    nc = _COMPILED["nc"]

    in_maps = _prep_host(
        message_bytes, initial_state, W1, b1, W2, b2, W3, b3)
    res = run_bass_kernel_spmd(nc, in_maps, list(range(NCORES)))
    out = np.concatenate([res.results[c]["out"] for c in range(NCORES)], axis=1)
    return np.ascontiguousarray(out.T)  # (B, 16) float32



# revision 13
# speedup vs baseline: 1.1440x; 1.1440x over previous
"""MD5Surrogate Bass kernel for 8x TRN2 NeuronCores.

Data-parallel over batch (2048 rows/core); 60 cheap fp8 rounds + 4 exact
fp32r tail rounds (the round recurrence contracts perturbations ~10x per
round, so early-round approximation error is invisible at the output).

Cheap-round structure:

1. The L3 matmul, +b3 epilogue and fp8 state roundtrip are folded into
   the NEXT round's L1:
       x_{r+1} = h2_r @ W3_r + b3_r
       L1_{r+1} = h2_r @ W13_{r+1} + words + const
   with W13_{r+1} = W3_r @ W1_{r+1}[0:16] precomputed host-side in fp8
   (rank 16, but PE time is cheap) and b3_r @ W1s_{r+1} folded into the
   const-row bias. L1 is a K=256 DoubleRow matmul over h2 plus a plain
   K=16 matmul over the msg word rows (msg8 is SBUF-resident).

2. Gelu is split across two engines running in parallel: the j0 feature
   half of h1/h2 is exact gelu on the scalar engine; the j1 half uses a
   custom fused DVE op (registered into dve_ops.OPS at import):
       out = t * clamp(s*t + 0.5, 0, 1),  t = in + bias
   one 1x DVE pass straight from PSUM to fp8 - no intermediate tiles,
   no weight re-scaling. Two phases per round (batch column pairs of
   1024) keep both engines and the PE pipelined across rounds.

The round-59 handoff (classic L3 -> fp32 xT) and the fp32r tail rounds
(60-63) are carried over from the previous kernel.
"""

import sys

sys.path.insert(0, "/opt/trn_rl_repo")

import numpy as np
import ml_dtypes

NUM_ROUNDS = 64
DH = 256
B = 16384
NCORES = 8
BC = B // NCORES          # batch per core = 2048
GW = 512                  # PSUM bank width (fp32) = matmul N per instruction
GPW = 1024                # column pair width (2 pairs)
SW = 1024                 # tail stream width
N_TAIL = 4
N_CHEAP = NUM_ROUNDS - N_TAIL
HANDOFF = N_CHEAP - 1     # last cheap round also writes the tail's x tiles

# gelu clamp approximation: y = t * clamp(S_GELU*t + B_GELU, 0, 1)
S_GELU = 0.2775
B_GELU = 0.5

SLAB_F = 804              # tail (fp32r) slab layout, same as the old kernel
WSL_F = 800               # cheap-round fp8 slab width

_SCHED = np.array(
    [i if i < 16 else ((5 * i + 1) % 16 if i < 32 else ((3 * i + 5) % 16 if i < 48 else (7 * i) % 16))
     for i in range(64)],
    dtype=np.int32,
)
_SHIFT = np.array(
    [7, 12, 17, 22] * 4 + [5, 9, 14, 20] * 4 + [4, 11, 16, 23] * 4 + [6, 10, 15, 21] * 4,
    dtype=np.float32,
)
_ROUND_INFO = np.stack(
    [np.arange(64, dtype=np.float32) / 64.0, _SHIFT / 25.0], axis=-1
)  # (64, 2)

F8NP = ml_dtypes.float8_e4m3

_COMPILED = {}
_DVE_OPS = {}


def _register_dve_ops():
    """Register the fused gelu-clamp ops into dve_ops.OPS (idempotent)."""
    if _DVE_OPS:
        return _DVE_OPS
    from concourse.dve_spec import Spec, Src0, C0, C1, C2, Zero, One, maxx, minn, lower
    from concourse.dve_spec import _has_src1 as has_src1
    from concourse import dve_ops
    from concourse.dve_uop import DveOpSpec

    def reg(name, spec):
        for op in dve_ops.OPS:
            if op.name == name:
                return op
        opcode = dve_ops._CUSTOM_DVE_ROW_BASE + len(dve_ops.OPS)
        shas = {}
        for ver in ("v3", "v4"):
            uops = lower(spec, ver=ver)
            s = DveOpSpec(name=name, opcode=opcode, uops=uops,
                          rd1_en=has_src1(spec))
            shas[ver] = s.sha(ver)
        op = dve_ops.DveOp(name, spec, False, shas)
        dve_ops.OPS.append(op)
        dve_ops._SUB_OPCODE_FOR_NAME[name] = opcode
        return op

    # out = in0 * clamp(in0*s0 + s1, 0, 1)           (no-bias variant, h1)
    _DVE_OPS["g"] = reg("ANT_GELU_CLAMP", Spec(
        body=Src0 * minn(maxx(Src0 * C0 + C1, Zero), One),
        reference=lambda in0, in1, s0, s1, imm2: (
            in0 * np.clip(in0 * s0 + s1, 0.0, 1.0)).astype(np.float32),
    ))
    # t = in0 + s0[P,1];  out = t * clamp(t*s1 + imm2, 0, 1)   (bias, h2)
    t = Src0 + C0
    _DVE_OPS["gb"] = reg("ANT_GELU_CLAMP_B", Spec(
        body=t * minn(maxx(t * C1 + C2, Zero), One),
        reference=lambda in0, in1, s0, s1, imm2: (
            (in0 + s0) * np.clip((in0 + s0) * s1 + imm2, 0.0, 1.0)
        ).astype(np.float32),
    ))
    return _DVE_OPS


def _build(time_loop_iters=0):
    import concourse.bass as bass  # noqa: F401
    from concourse import bacc
    import concourse.mybir as mybir
    from concourse.tile import TileContext

    dops = _register_dve_ops()

    F32 = mybir.dt.float32
    F32R = mybir.dt.float32r
    F8 = mybir.dt.float8e4
    BF16 = mybir.dt.bfloat16
    AF = mybir.ActivationFunctionType
    DR = mybir.MatmulPerfMode.DoubleRow
    ALU = mybir.AluOpType

    nc = bacc.Bacc()
    st8_d = nc.dram_tensor("st8", [128, GPW], F8, kind="ExternalInput")
    msg8_d = nc.dram_tensor("msg8", [128, 16, GPW], F8, kind="ExternalInput")
    wsl8_d = nc.dram_tensor("wsl8", [N_CHEAP, 128, 2, WSL_F], F8, kind="ExternalInput")
    bias8_d = nc.dram_tensor("bias8", [128, 2 * N_CHEAP + 1], F32, kind="ExternalInput")
    msgt_d = nc.dram_tensor("msgt", [65, BC], F32R, kind="ExternalInput")
    slabt_d = nc.dram_tensor("slabt", [N_TAIL, 128, SLAB_F], F32R, kind="ExternalInput")
    biast_d = nc.dram_tensor("biast", [128, 4 * N_TAIL], F32, kind="ExternalInput")
    out_d = nc.dram_tensor("out", [16, BC], F32, kind="ExternalOutput")

    B3H = 2 * N_CHEAP  # col of bias8 holding the handoff round's b3

    with TileContext(nc) as tc:
        with tc.tile_pool(name="cpool", bufs=1) as cpool, \
             tc.tile_pool(name="wpool", bufs=6) as wpool, \
             tc.tile_pool(name="pspool", bufs=4, space="PSUM") as ps:

            x0 = cpool.tile([128, 2, GPW], F8, name="x0")
            msg8 = cpool.tile([128, 16, GPW], F8, name="msg8")
            h1 = cpool.tile([128, 2, BC], F8, name="h1")
            h2 = cpool.tile([128, 2, BC], F8, name="h2")
            b8 = cpool.tile([128, 2 * N_CHEAP + 1], F32, name="b8")
            bT = cpool.tile([128, 4 * N_TAIL], F32, name="bT")
            xT = [cpool.tile([21, SW], F32R, name=f"xT{s}") for s in range(2)]
            h1T = [cpool.tile([128, 2 * SW], F32R, name=f"h1T{s}") for s in range(2)]
            h2T = [cpool.tile([128, 2 * SW], F32R, name=f"h2T{s}") for s in range(2)]
            outt = cpool.tile([16, BC], F32, name="outt")

            # prologue DMAs
            nc.sync.dma_start(x0[:, 0, :], st8_d[:])
            nc.sync.dma_start(x0[:, 1, :], msg8_d[:, int(_SCHED[0]), :])
            nc.sync.dma_start(msg8[:], msg8_d[:])
            nc.sync.dma_start(b8[:], bias8_d[:])
            nc.sync.dma_start(bT[:], biast_d[:])
            for s in range(2):
                nc.sync.dma_start(xT[s][16:17, :], msgt_d[64:65, s * SW:(s + 1) * SW])

            wTs = []

            # --- software-pipelined cheap phases -------------------------
            # One phase = one 1024-column pair p of one round r (ph = 2r+p).
            # Within each stage the j0 feature half runs exact gelu on ACT
            # while the j1 half runs the fused clamp op on DVE - the two
            # engines work in parallel and the per-pair round latency
            # hides under the 2-phase pipeline.
            PH = {}        # ph -> dict with tiles/metadata
            w8s = {}       # round -> weight slab tile

            def phase_front(ph):
                """Slab DMA + L1 matmuls for phase ph."""
                r, p = divmod(ph, 2)
                if p == 0:
                    w8 = wpool.tile([128, 2, WSL_F], F8, tag="w8", name="w8")
                    w8s[r] = w8
                    nc.sync.dma_start(w8[:], wsl8_d[r])
                    if N_CHEAP - 9 <= r < N_CHEAP - 9 + N_TAIL:
                        t = r - (N_CHEAP - 9)
                        wT = wpool.tile([128, SLAB_F], F32R, tag="wT",
                                        bufs=N_TAIL, name="wT")
                        wTs.append(wT)
                        nc.sync.dma_start(wT[:], slabt_d[t])
                    if r >= 2:
                        del w8s[r - 2]
                w8 = w8s[r]
                st = PH[ph] = {"r": r, "p": p, "w8": w8}
                lo = p * GPW
                ps1 = [ps.tile([128, GPW], F32, tag="big", name=f"ps1_{ph}_{j}")
                       for j in range(2)]
                st["ps1"] = ps1
                g = int(_SCHED[r])
                for j in (1, 0):
                    for c in range(2):
                        cs = slice(c * GW, (c + 1) * GW)
                        gcs = slice(lo + c * GW, lo + (c + 1) * GW)
                        if r == 0:
                            # classic L1: DR over state+word planes (K=16x2)
                            nc.tensor.matmul(
                                ps1[j][:, cs],
                                w8[64 * p:64 * p + 16, :, 512 + j * 128:512 + (j + 1) * 128],
                                x0[64 * p:64 * p + 16, :, cs],
                                start=True, stop=True, perf_mode=DR)
                        else:
                            # word/const plane: plain K=16 matmul from msg8
                            nc.tensor.matmul(
                                ps1[j][:, cs],
                                w8[64 * p:64 * p + 16, 0, 512 + j * 128:512 + (j + 1) * 128],
                                msg8[64 * p:64 * p + 16, g, cs],
                                start=True, stop=False)
                            # h2(r-1) @ W13: DR over K=256
                            nc.tensor.matmul(
                                ps1[j][:, cs],
                                w8[:, :, 256 + j * 128:256 + (j + 1) * 128],
                                h2[:, :, gcs],
                                start=False, stop=True, perf_mode=DR)

            def h1ops(ph):
                """h1 activations: j1 fused on DVE, j0 exact on ACT."""
                st = PH[ph]
                ps1, p = st["ps1"], st["p"]
                lo = p * GPW
                nc.vector._custom_dve(
                    dops["g"], out=h1[:, 1, lo:lo + GPW], in0=ps1[1][:],
                    s0=S_GELU, s1=B_GELU)
                nc.scalar.activation(h1[:, 0, lo:lo + GPW], ps1[0][:], AF.Gelu)

            def phase_mid1(ph):
                """L2 for phase ph (reuses the L1 psum tiles: L2 must wait
                for h1's reads of ps1 anyway, and 2 tiles/phase leaves room
                for two phases in flight across the 8 PSUM banks)."""
                st = PH[ph]
                p, w8 = st["p"], st["w8"]
                lo = p * GPW
                ps2 = st["ps1"]
                st["ps2"] = ps2
                for j in (1, 0):
                    for c in range(2):
                        nc.tensor.matmul(
                            ps2[j][:, c * GW:(c + 1) * GW],
                            w8[:, :, j * 128:(j + 1) * 128],
                            h1[:, :, lo + c * GW:lo + (c + 1) * GW],
                            start=True, stop=True, perf_mode=DR)

            def phase_mid2(ph):
                """h2 activations: j1 fused on DVE (bias AP), j0 on ACT."""
                st = PH[ph]
                r, p = st["r"], st["p"]
                ps2 = st["ps2"]
                lo = p * GPW
                nc.vector._custom_dve(
                    dops["gb"], out=h2[:, 1, lo:lo + GPW], in0=ps2[1][:],
                    s0=b8[:, 2 * r + 1:2 * r + 2], s1=S_GELU, imm2=B_GELU)
                nc.scalar.activation(
                    h2[:, 0, lo:lo + GPW], ps2[0][:], AF.Gelu,
                    bias=b8[:, 2 * r:2 * r + 1])

            def phase_back(ph):
                """Handoff-round L3 + xT epilogue (classic, M=16)."""
                st = PH[ph]
                p, w8, ps2 = st["p"], st["w8"], st["ps2"]
                for c in range(2):
                    cg = 2 * p + c
                    sl = ps2[1][0:16, c * GW:(c + 1) * GW]
                    nc.tensor.matmul(
                        sl, w8[:, :, 768:784],
                        h2[:, :, cg * GW:(cg + 1) * GW],
                        start=True, stop=True, perf_mode=DR)
                    nc.vector.tensor_scalar(
                        xT[p][0:16, c * GW:(c + 1) * GW], sl,
                        b8[0:16, B3H:B3H + 1], None, ALU.add)

            def tail_round(t, is_last):
                slab = wTs[t]
                W2v = slab[:, 0:512]
                W3v = slab[:, 512:544]
                W1v = slab[0:21, 547:803]
                g = int(_SCHED[N_CHEAP + t])
                for s in range(2):
                    nc.sync.dma_start(
                        xT[s][17:21, :], msgt_d[4 * g:4 * g + 4, s * SW:(s + 1) * SW])
                pt1 = {}
                for s in range(2):
                    for j in range(2):
                        p1 = ps.tile([128, GPW], F32, tag="big", name=f"pt1_{s}{j}")
                        pt1[s, j] = p1
                        for b in range(2):
                            nc.tensor.matmul(
                                p1[:, b * 512:(b + 1) * 512],
                                W1v[:, j * 128:(j + 1) * 128],
                                xT[s][:, b * 512:(b + 1) * 512],
                                start=True, stop=True)
                        nc.scalar.activation(
                            h1T[s][:, j * SW:(j + 1) * SW], p1[:], AF.Gelu)
                for s in range(2):
                    pt2 = {}
                    for j in range(2):
                        p2 = pt1[s, j]
                        pt2[j] = p2
                        for b in range(2):
                            for k in range(2):
                                nc.tensor.matmul(
                                    p2[:, b * 512:(b + 1) * 512],
                                    W2v[:, (2 * k + j) * 128:(2 * k + j + 1) * 128],
                                    h1T[s][:, k * SW + b * 512:k * SW + (b + 1) * 512],
                                    start=(k == 0), stop=(k == 1))
                        nc.scalar.activation(
                            h2T[s][:, j * SW:(j + 1) * SW], p2[:], AF.Gelu,
                            bias=bT[:, 4 * t + j:4 * t + j + 1])
                    for b in range(2):
                        sl = pt2[1][0:16, b * 512:(b + 1) * 512]
                        for k in range(2):
                            nc.tensor.matmul(
                                sl, W3v[:, k * 16:(k + 1) * 16],
                                h2T[s][:, k * SW + b * 512:k * SW + (b + 1) * 512],
                                start=(k == 0), stop=(k == 1))
                        b3v = bT[0:16, 4 * t + 2:4 * t + 3]
                        if is_last:
                            nc.vector.tensor_scalar(
                                outt[:, s * SW + b * 512:s * SW + (b + 1) * 512],
                                sl, b3v, None, ALU.add)
                        else:
                            nc.vector.tensor_scalar(
                                xT[s][0:16, b * 512:(b + 1) * 512], sl, b3v,
                                None, ALU.add)

            def whole_pass():
                NP = 2 * N_CHEAP
                for ph in range(NP):
                    phase_front(ph)
                    if ph >= 1:
                        phase_mid1(ph - 1)
                        phase_mid2(ph - 1)
                    h1ops(ph)
                    if ph >= 1 and (ph - 1) // 2 == HANDOFF:
                        phase_back(ph - 1)
                phase_mid1(NP - 1)
                phase_mid2(NP - 1)
                phase_back(NP - 1)
                PH.clear()
                for t in range(N_TAIL):
                    tail_round(t, t == N_TAIL - 1)
                wTs.clear()
                w8s.clear()

            if time_loop_iters:
                with tc.For_i(0, time_loop_iters, 1):
                    whole_pass()
            else:
                whole_pass()

            nc.sync.dma_start(out_d[:], outt[:])

    nc.compile()
    return nc


def _prep_host(message_bytes, initial_state, W1, b1, W2, b2, W3, b3):
    """Build per-core input tensors. Returns a list of dicts (one per core)."""
    message_bytes = np.asarray(message_bytes, dtype=np.float32)
    initial_state = np.asarray(initial_state, dtype=np.float32)
    W1 = np.asarray(W1, dtype=np.float32)
    b1 = np.asarray(b1, dtype=np.float32)
    W2 = np.asarray(W2, dtype=np.float32)
    b2 = np.asarray(b2, dtype=np.float32)
    W3 = np.asarray(W3, dtype=np.float32)
    b3 = np.asarray(b3, dtype=np.float32)

    # ---- shared (weight) tensors ----
    # cheap-round fp8 slab [N_CHEAP, 128, 2, 800]:
    #   [0:256)   W2pack
    #   [256:512) W13pack = W3_{i-1} @ W1s_i  (K=256 DR over h2)
    #   [512:768) word plane: k=0 rows 64g+0 = b1p', 64g+1..4 = W1[16:20]
    #             (round 0 instead: k=0 state W1[0:16], k=1 word plane)
    #   [768:784) W3pack plain (handoff round only)
    wsl8 = np.zeros((N_CHEAP, 128, 2, WSL_F), dtype=np.float32)
    bias8 = np.zeros((128, 2 * N_CHEAP + 1), dtype=np.float32)
    pidx = np.arange(128)
    for i in range(N_CHEAP):
        for k in range(2):
            wsl8[i, :, k, 0:256] = W2[i][k * 128 + pidx, :]
        b1p = b1[i] + _ROUND_INFO[i] @ W1[i][20:22]
        if i >= 1:
            W13 = W3[i - 1] @ W1[i][0:16]   # (256, 256)
            for k in range(2):
                wsl8[i, :, k, 256:512] = W13[k * 128 + pidx, :]
            b1p = b1p + b3[i - 1] @ W1[i][0:16]
        for g in range(2):
            if i == 0:
                sl = slice(64 * g, 64 * g + 16)
                wsl8[0, sl, 0, 512:768] = W1[0][0:16]
                wsl8[0, 64 * g + 0, 1, 512:768] = b1p
                wsl8[0, 64 * g + 1:64 * g + 5, 1, 512:768] = W1[0][16:20]
            else:
                wsl8[i, 64 * g + 0, 0, 512:768] = b1p
                wsl8[i, 64 * g + 1:64 * g + 5, 0, 512:768] = W1[i][16:20]
        bias8[:, 2 * i + 0] = b2[i][0:128]
        bias8[:, 2 * i + 1] = b2[i][128:256]
    for k in range(2):
        wsl8[HANDOFF, :, k, 768:784] = W3[HANDOFF][k * 128 + pidx, :]
    bias8[0:16, 2 * N_CHEAP] = b3[HANDOFF]
    wsl8 = wsl8.astype(F8NP)

    # tail fp32 slab (same layout as the old all-fp32r kernel)
    slabt = np.zeros((N_TAIL, 128, SLAB_F), dtype=np.float32)
    biast = np.zeros((128, 4 * N_TAIL), dtype=np.float32)
    for t in range(N_TAIL):
        i = N_CHEAP + t
        W2i = W2[i]
        slabt[t, :, 0:128] = W2i[0:128, 0:128]
        slabt[t, :, 128:256] = W2i[0:128, 128:256]
        slabt[t, :, 256:384] = W2i[128:256, 0:128]
        slabt[t, :, 384:512] = W2i[128:256, 128:256]
        W3i = W3[i]
        slabt[t, :, 512:528] = W3i[0:128, :]
        slabt[t, :, 528:544] = W3i[128:256, :]
        biast[:, 4 * t + 0] = b2[i][0:128]
        biast[:, 4 * t + 1] = b2[i][128:256]
        biast[0:16, 4 * t + 2] = b3[i]
        b1p = b1[i] + _ROUND_INFO[i] @ W1[i][20:22]
        slabt[t, 0:16, 547:803] = W1[i][0:16]
        slabt[t, 16, 547:803] = b1p
        slabt[t, 17:21, 547:803] = W1[i][16:20]

    # ---- per-core (batch-sharded) tensors ----
    in_maps = []
    for c in range(NCORES):
        cols = slice(c * BC, (c + 1) * BC)
        mb = message_bytes[cols]          # (BC, 64)
        st = initial_state[cols]          # (BC, 16)
        st8 = np.zeros((128, GPW), dtype=np.float32)
        msg8 = np.zeros((128, 16, GPW), dtype=np.float32)
        for g in range(2):
            gc = slice(g * GPW, (g + 1) * GPW)
            st8[64 * g:64 * g + 16, :] = st[gc].T
            msg8[64 * g + 0, :, :] = 1.0
            for k in range(4):
                msg8[64 * g + 1 + k, :, :] = mb[gc, k::4].T  # word w, byte k
        msgt = np.empty((65, BC), dtype=np.float32)
        msgt[0:64] = mb.T
        msgt[64] = 1.0
        in_maps.append({
            "st8": st8.astype(F8NP),
            "msg8": msg8.astype(F8NP),
            "wsl8": wsl8,
            "bias8": bias8,
            "msgt": msgt,
            "slabt": slabt,
            "biast": biast,
        })
    return in_maps


def kernel(message_bytes, initial_state, W1, b1, W2, b2, W3, b3):
    from concourse.bass_utils import run_bass_kernel_spmd

    if "nc" not in _COMPILED:
        _COMPILED["nc"] = _build()
    nc = _COMPILED["nc"]

    in_maps = _prep_host(
        message_bytes, initial_state, W1, b1, W2, b2, W3, b3)
    res = run_bass_kernel_spmd(nc, in_maps, list(range(NCORES)))
    out = np.concatenate([res.results[c]["out"] for c in range(NCORES)], axis=1)
    return np.ascontiguousarray(out.T)  # (B, 16) float32


# revision 15
# speedup vs baseline: 1.6928x; 1.4798x over previous
"""MD5Surrogate Bass kernel for 8x TRN2 NeuronCores.

Data-parallel over batch (2048 rows/core); 60 cheap fp8 rounds + 4 exact
fp32r tail rounds (the round recurrence contracts perturbations ~10x per
round, so early-round approximation error is invisible at the output).

Cheap-round structure:

1. The L3 matmul, +b3 epilogue and fp8 state roundtrip are folded into
   the NEXT round's L1:
       x_{r+1} = h2_r @ W3_r + b3_r
       L1_{r+1} = h2_r @ W13_{r+1} + words + const
   with W13_{r+1} = W3_r @ W1_{r+1}[0:16] precomputed host-side in fp8
   (rank 16, but PE time is cheap) and b3_r @ W1s_{r+1} folded into the
   const-row bias. L1 is a K=256 DoubleRow matmul over h2 plus a plain
   K=16 matmul over the msg word rows (msg8 is SBUF-resident).

2. Gelu is split across two engines running in parallel: the j0 feature
   half of h1/h2 is exact gelu on the scalar engine; the j1 half uses a
   custom fused DVE op (registered into dve_ops.OPS at import):
       out = t * clamp(s*t + 0.5, 0, 1),  t = in + bias
   one 1x DVE pass straight from PSUM to fp8 - no intermediate tiles,
   no weight re-scaling. Two phases per round (batch column pairs of
   1024) keep both engines and the PE pipelined across rounds.

The round-59 handoff (classic L3 -> fp32 xT) and the fp32r tail rounds
(60-63) are carried over from the previous kernel.
"""

import sys

sys.path.insert(0, "/opt/trn_rl_repo")

import numpy as np
import ml_dtypes

NUM_ROUNDS = 64
DH = 256
B = 16384
NCORES = 8
BC = B // NCORES          # batch per core = 2048
GW = 512                  # PSUM bank width (fp32) = matmul N per instruction
GPW = 1024                # column pair width (2 pairs)
SW = 1024                 # tail stream width
N_TAIL = 4
N_CHEAP = NUM_ROUNDS - N_TAIL
HANDOFF = N_CHEAP - 1     # last cheap round also writes the tail's x tiles

# gelu clamp approximation: y = t * clamp(S_GELU*t + B_GELU, 0, 1)
S_GELU = 0.2775
B_GELU = 0.5

SLAB_F = 804              # tail (fp32r) slab layout, same as the old kernel
WSL_F = 800               # cheap-round fp8 slab width

_SCHED = np.array(
    [i if i < 16 else ((5 * i + 1) % 16 if i < 32 else ((3 * i + 5) % 16 if i < 48 else (7 * i) % 16))
     for i in range(64)],
    dtype=np.int32,
)
_SHIFT = np.array(
    [7, 12, 17, 22] * 4 + [5, 9, 14, 20] * 4 + [4, 11, 16, 23] * 4 + [6, 10, 15, 21] * 4,
    dtype=np.float32,
)
_ROUND_INFO = np.stack(
    [np.arange(64, dtype=np.float32) / 64.0, _SHIFT / 25.0], axis=-1
)  # (64, 2)

F8NP = ml_dtypes.float8_e4m3

_COMPILED = {}
_DVE_OPS = {}


def _register_dve_ops():
    """Register the fused gelu-clamp ops into dve_ops.OPS (idempotent)."""
    if _DVE_OPS:
        return _DVE_OPS
    from concourse.dve_spec import Spec, Src0, C0, C1, C2, Zero, One, maxx, minn, lower
    from concourse.dve_spec import _has_src1 as has_src1
    from concourse import dve_ops
    from concourse.dve_uop import DveOpSpec

    def reg(name, spec):
        for op in dve_ops.OPS:
            if op.name == name:
                return op
        opcode = dve_ops._CUSTOM_DVE_ROW_BASE + len(dve_ops.OPS)
        shas = {}
        for ver in ("v3", "v4"):
            uops = lower(spec, ver=ver)
            s = DveOpSpec(name=name, opcode=opcode, uops=uops,
                          rd1_en=has_src1(spec))
            shas[ver] = s.sha(ver)
        op = dve_ops.DveOp(name, spec, False, shas)
        dve_ops.OPS.append(op)
        dve_ops._SUB_OPCODE_FOR_NAME[name] = opcode
        return op

    # out = in0 * clamp(in0*s0 + s1, 0, 1)           (no-bias variant, h1)
    _DVE_OPS["g"] = reg("ANT_GELU_CLAMP", Spec(
        body=Src0 * minn(maxx(Src0 * C0 + C1, Zero), One),
        reference=lambda in0, in1, s0, s1, imm2: (
            in0 * np.clip(in0 * s0 + s1, 0.0, 1.0)).astype(np.float32),
    ))
    # t = in0 + s0[P,1];  out = t * clamp(t*s1 + imm2, 0, 1)   (bias, h2)
    t = Src0 + C0
    _DVE_OPS["gb"] = reg("ANT_GELU_CLAMP_B", Spec(
        body=t * minn(maxx(t * C1 + C2, Zero), One),
        reference=lambda in0, in1, s0, s1, imm2: (
            (in0 + s0) * np.clip((in0 + s0) * s1 + imm2, 0.0, 1.0)
        ).astype(np.float32),
    ))
    return _DVE_OPS


def _build(time_loop_iters=0):
    import concourse.bass as bass  # noqa: F401
    from concourse import bacc
    import concourse.mybir as mybir
    from concourse.tile import TileContext

    dops = _register_dve_ops()

    F32 = mybir.dt.float32
    F32R = mybir.dt.float32r
    F8 = mybir.dt.float8e4
    BF16 = mybir.dt.bfloat16
    AF = mybir.ActivationFunctionType
    DR = mybir.MatmulPerfMode.DoubleRow
    ALU = mybir.AluOpType

    nc = bacc.Bacc()
    st8_d = nc.dram_tensor("st8", [128, GPW], F8, kind="ExternalInput")
    msg8_d = nc.dram_tensor("msg8", [128, 16, GPW], F8, kind="ExternalInput")
    w0_d = nc.dram_tensor("w0", [128, GPW], F8, kind="ExternalInput")
    wsl8_d = nc.dram_tensor("wsl8", [N_CHEAP, 128, 2, WSL_F], F8, kind="ExternalInput")
    bias8_d = nc.dram_tensor("bias8", [128, 2 * N_CHEAP + 1], F32, kind="ExternalInput")
    msgt_d = nc.dram_tensor("msgt", [65, BC], F32R, kind="ExternalInput")
    slabt_d = nc.dram_tensor("slabt", [N_TAIL, 128, SLAB_F], F32R, kind="ExternalInput")
    biast_d = nc.dram_tensor("biast", [128, 4 * N_TAIL], F32, kind="ExternalInput")
    out_d = nc.dram_tensor("out", [16, BC], F32, kind="ExternalOutput")

    B3H = 2 * N_CHEAP  # col of bias8 holding the handoff round's b3

    with TileContext(nc) as tc:
        with tc.tile_pool(name="cpool", bufs=1) as cpool, \
             tc.tile_pool(name="wpool", bufs=6) as wpool, \
             tc.tile_pool(name="pspool", bufs=4, space="PSUM") as ps:

            x0 = cpool.tile([128, 2, GPW], F8, name="x0")
            msg8 = cpool.tile([128, 16, GPW], F8, name="msg8")
            h1 = cpool.tile([128, 2, BC], F8, name="h1")
            h2 = cpool.tile([128, 2, BC], F8, name="h2")
            b8 = cpool.tile([128, 2 * N_CHEAP + 1], F32, name="b8")
            bT = cpool.tile([128, 4 * N_TAIL], F32, name="bT")
            xT = [cpool.tile([21, SW], F32R, name=f"xT{s}") for s in range(2)]
            h1T = [cpool.tile([128, 2 * SW], F32R, name=f"h1T{s}") for s in range(2)]
            h2T = [cpool.tile([128, 2 * SW], F32R, name=f"h2T{s}") for s in range(2)]
            outt = cpool.tile([16, BC], F32, name="outt")

            # prologue DMAs
            nc.sync.dma_start(x0[:, 0, :], st8_d[:])
            nc.sync.dma_start(x0[:, 1, :], w0_d[:])
            nc.sync.dma_start(msg8[:], msg8_d[:])
            nc.sync.dma_start(b8[:], bias8_d[:])
            nc.sync.dma_start(bT[:], biast_d[:])
            for s in range(2):
                nc.sync.dma_start(xT[s][16:17, :], msgt_d[64:65, s * SW:(s + 1) * SW])

            wTs = []

            # --- software-pipelined cheap phases -------------------------
            # One phase = one 1024-column pair p of one round r (ph = 2r+p).
            # Within each stage the j0 feature half runs exact gelu on ACT
            # while the j1 half runs the fused clamp op on DVE - the two
            # engines work in parallel and the per-pair round latency
            # hides under the 2-phase pipeline.
            PH = {}        # ph -> dict with tiles/metadata
            w8s = {}       # round -> weight slab tile

            def phase_front(ph):
                """Slab DMA + L1 matmuls for phase ph."""
                r, p = divmod(ph, 2)
                if p == 0:
                    w8 = wpool.tile([128, 2, WSL_F], F8, tag="w8", name="w8")
                    w8s[r] = w8
                    nc.sync.dma_start(w8[:], wsl8_d[r])
                    if N_CHEAP - 9 <= r < N_CHEAP - 9 + N_TAIL:
                        t = r - (N_CHEAP - 9)
                        wT = wpool.tile([128, SLAB_F], F32R, tag="wT",
                                        bufs=N_TAIL, name="wT")
                        wTs.append(wT)
                        nc.sync.dma_start(wT[:], slabt_d[t])
                    if r >= 2:
                        del w8s[r - 2]
                w8 = w8s[r]
                st = PH[ph] = {"r": r, "p": p, "w8": w8}
                lo = p * GPW
                ps1 = [ps.tile([128, GPW], F32, tag="big", name=f"ps1_{ph}_{j}")
                       for j in range(2)]
                st["ps1"] = ps1
                g = int(_SCHED[r])
                if r == 0:
                    # classic L1: DR over state+word planes (K=16x2)
                    for j in (1, 0):
                        for c in range(2):
                            cs = slice(c * GW, (c + 1) * GW)
                            nc.tensor.matmul(
                                ps1[j][:, cs],
                                w8[64 * p:64 * p + 16, :, 512 + j * 128:512 + (j + 1) * 128],
                                x0[64 * p:64 * p + 16, :, cs],
                                start=True, stop=True, perf_mode=DR)
                else:
                    # word/const plane: 4 concurrent row-tiled K=16 matmuls
                    # (strip s = 2j+c; msg8 holds the pair-p chunk at
                    # columns p*512 of strip s rows)
                    for j in (1, 0):
                        for c in range(2):
                            s = 2 * j + c
                            nc.tensor.matmul(
                                ps1[j][:, c * GW:(c + 1) * GW],
                                w8[32 * s:32 * s + 16, 0, 512 + j * 128:512 + (j + 1) * 128],
                                msg8[32 * s:32 * s + 16, g, p * GW:(p + 1) * GW],
                                start=True, stop=False,
                                tile_position=(32 * s, 0))
                    # h2(r-1) @ W13: DR over K=256
                    for j in (1, 0):
                        for c in range(2):
                            cs = slice(c * GW, (c + 1) * GW)
                            gcs = slice(lo + c * GW, lo + (c + 1) * GW)
                            nc.tensor.matmul(
                                ps1[j][:, cs],
                                w8[:, :, 256 + j * 128:256 + (j + 1) * 128],
                                h2[:, :, gcs],
                                start=False, stop=True, perf_mode=DR)

            def h1ops(ph):
                """h1 activations: j1 fused on DVE, j0 exact on ACT."""
                st = PH[ph]
                ps1, p = st["ps1"], st["p"]
                lo = p * GPW
                nc.vector._custom_dve(
                    dops["g"], out=h1[:, 1, lo:lo + GPW], in0=ps1[1][:],
                    s0=S_GELU, s1=B_GELU)
                nc.scalar.activation(h1[:, 0, lo:lo + GPW], ps1[0][:], AF.Gelu)

            def phase_mid1(ph):
                """L2 for phase ph (reuses the L1 psum tiles: L2 must wait
                for h1's reads of ps1 anyway, and 2 tiles/phase leaves room
                for two phases in flight across the 8 PSUM banks)."""
                st = PH[ph]
                p, w8 = st["p"], st["w8"]
                lo = p * GPW
                ps2 = st["ps1"]
                st["ps2"] = ps2
                for j in (1, 0):
                    for c in range(2):
                        nc.tensor.matmul(
                            ps2[j][:, c * GW:(c + 1) * GW],
                            w8[:, :, j * 128:(j + 1) * 128],
                            h1[:, :, lo + c * GW:lo + (c + 1) * GW],
                            start=True, stop=True, perf_mode=DR)

            def phase_mid2(ph):
                """h2 activations: j1 fused on DVE (bias AP), j0 on ACT."""
                st = PH[ph]
                r, p = st["r"], st["p"]
                ps2 = st["ps2"]
                lo = p * GPW
                nc.vector._custom_dve(
                    dops["gb"], out=h2[:, 1, lo:lo + GPW], in0=ps2[1][:],
                    s0=b8[:, 2 * r + 1:2 * r + 2], s1=S_GELU, imm2=B_GELU)
                nc.scalar.activation(
                    h2[:, 0, lo:lo + GPW], ps2[0][:], AF.Gelu,
                    bias=b8[:, 2 * r:2 * r + 1])

            def phase_back(ph):
                """Handoff-round L3 + xT epilogue (classic, M=16)."""
                st = PH[ph]
                p, w8, ps2 = st["p"], st["w8"], st["ps2"]
                for c in range(2):
                    cg = 2 * p + c
                    sl = ps2[1][0:16, c * GW:(c + 1) * GW]
                    nc.tensor.matmul(
                        sl, w8[:, :, 768:784],
                        h2[:, :, cg * GW:(cg + 1) * GW],
                        start=True, stop=True, perf_mode=DR)
                    nc.vector.tensor_scalar(
                        xT[p][0:16, c * GW:(c + 1) * GW], sl,
                        b8[0:16, B3H:B3H + 1], None, ALU.add)

            def tail_round(t, is_last):
                slab = wTs[t]
                W2v = slab[:, 0:512]
                W3v = slab[:, 512:544]
                W1v = slab[0:21, 547:803]
                g = int(_SCHED[N_CHEAP + t])
                for s in range(2):
                    nc.sync.dma_start(
                        xT[s][17:21, :], msgt_d[4 * g:4 * g + 4, s * SW:(s + 1) * SW])
                pt1 = {}
                for s in range(2):
                    for j in range(2):
                        p1 = ps.tile([128, GPW], F32, tag="big", name=f"pt1_{s}{j}")
                        pt1[s, j] = p1
                        for b in range(2):
                            nc.tensor.matmul(
                                p1[:, b * 512:(b + 1) * 512],
                                W1v[:, j * 128:(j + 1) * 128],
                                xT[s][:, b * 512:(b + 1) * 512],
                                start=True, stop=True)
                        nc.scalar.activation(
                            h1T[s][:, j * SW:(j + 1) * SW], p1[:], AF.Gelu)
                for s in range(2):
                    pt2 = {}
                    for j in range(2):
                        p2 = pt1[s, j]
                        pt2[j] = p2
                        for b in range(2):
                            for k in range(2):
                                nc.tensor.matmul(
                                    p2[:, b * 512:(b + 1) * 512],
                                    W2v[:, (2 * k + j) * 128:(2 * k + j + 1) * 128],
                                    h1T[s][:, k * SW + b * 512:k * SW + (b + 1) * 512],
                                    start=(k == 0), stop=(k == 1))
                        nc.scalar.activation(
                            h2T[s][:, j * SW:(j + 1) * SW], p2[:], AF.Gelu,
                            bias=bT[:, 4 * t + j:4 * t + j + 1])
                    for b in range(2):
                        sl = pt2[1][0:16, b * 512:(b + 1) * 512]
                        for k in range(2):
                            nc.tensor.matmul(
                                sl, W3v[:, k * 16:(k + 1) * 16],
                                h2T[s][:, k * SW + b * 512:k * SW + (b + 1) * 512],
                                start=(k == 0), stop=(k == 1))
                        b3v = bT[0:16, 4 * t + 2:4 * t + 3]
                        if is_last:
                            nc.vector.tensor_scalar(
                                outt[:, s * SW + b * 512:s * SW + (b + 1) * 512],
                                sl, b3v, None, ALU.add)
                        else:
                            nc.vector.tensor_scalar(
                                xT[s][0:16, b * 512:(b + 1) * 512], sl, b3v,
                                None, ALU.add)

            def whole_pass():
                NP = 2 * N_CHEAP
                for ph in range(NP):
                    phase_front(ph)
                    if ph >= 1:
                        phase_mid1(ph - 1)
                        phase_mid2(ph - 1)
                    h1ops(ph)
                    if ph >= 1 and (ph - 1) // 2 == HANDOFF:
                        phase_back(ph - 1)
                phase_mid1(NP - 1)
                phase_mid2(NP - 1)
                phase_back(NP - 1)
                PH.clear()
                for t in range(N_TAIL):
                    tail_round(t, t == N_TAIL - 1)
                wTs.clear()
                w8s.clear()

            if time_loop_iters:
                with tc.For_i(0, time_loop_iters, 1):
                    whole_pass()
            else:
                whole_pass()

            nc.sync.dma_start(out_d[:], outt[:])

    nc.compile()
    return nc


def _prep_host(message_bytes, initial_state, W1, b1, W2, b2, W3, b3):
    """Build per-core input tensors. Returns a list of dicts (one per core)."""
    message_bytes = np.asarray(message_bytes, dtype=np.float32)
    initial_state = np.asarray(initial_state, dtype=np.float32)
    W1 = np.asarray(W1, dtype=np.float32)
    b1 = np.asarray(b1, dtype=np.float32)
    W2 = np.asarray(W2, dtype=np.float32)
    b2 = np.asarray(b2, dtype=np.float32)
    W3 = np.asarray(W3, dtype=np.float32)
    b3 = np.asarray(b3, dtype=np.float32)

    # ---- shared (weight) tensors ----
    # cheap-round fp8 slab [N_CHEAP, 128, 2, 800]:
    #   [0:256)   W2pack
    #   [256:512) W13pack = W3_{i-1} @ W1s_i  (K=256 DR over h2)
    #   [512:768) word plane: k=0 rows 64g+0 = b1p', 64g+1..4 = W1[16:20]
    #             (round 0 instead: k=0 state W1[0:16], k=1 word plane)
    #   [768:784) W3pack plain (handoff round only)
    wsl8 = np.zeros((N_CHEAP, 128, 2, WSL_F), dtype=np.float32)
    bias8 = np.zeros((128, 2 * N_CHEAP + 1), dtype=np.float32)
    pidx = np.arange(128)
    for i in range(N_CHEAP):
        for k in range(2):
            wsl8[i, :, k, 0:256] = W2[i][k * 128 + pidx, :]
        b1p = b1[i] + _ROUND_INFO[i] @ W1[i][20:22]
        if i >= 1:
            W13 = W3[i - 1] @ W1[i][0:16]   # (256, 256)
            for k in range(2):
                wsl8[i, :, k, 256:512] = W13[k * 128 + pidx, :]
            b1p = b1p + b3[i - 1] @ W1[i][0:16]
        if i == 0:
            for g in range(2):
                sl = slice(64 * g, 64 * g + 16)
                wsl8[0, sl, 0, 512:768] = W1[0][0:16]
                wsl8[0, 64 * g + 0, 1, 512:768] = b1p
                wsl8[0, 64 * g + 1:64 * g + 5, 1, 512:768] = W1[0][16:20]
        else:
            for s in range(4):
                wsl8[i, 32 * s + 0, 0, 512:768] = b1p
                wsl8[i, 32 * s + 1:32 * s + 5, 0, 512:768] = W1[i][16:20]
        bias8[:, 2 * i + 0] = b2[i][0:128]
        bias8[:, 2 * i + 1] = b2[i][128:256]
    for k in range(2):
        wsl8[HANDOFF, :, k, 768:784] = W3[HANDOFF][k * 128 + pidx, :]
    bias8[0:16, 2 * N_CHEAP] = b3[HANDOFF]
    wsl8 = wsl8.astype(F8NP)

    # tail fp32 slab (same layout as the old all-fp32r kernel)
    slabt = np.zeros((N_TAIL, 128, SLAB_F), dtype=np.float32)
    biast = np.zeros((128, 4 * N_TAIL), dtype=np.float32)
    for t in range(N_TAIL):
        i = N_CHEAP + t
        W2i = W2[i]
        slabt[t, :, 0:128] = W2i[0:128, 0:128]
        slabt[t, :, 128:256] = W2i[0:128, 128:256]
        slabt[t, :, 256:384] = W2i[128:256, 0:128]
        slabt[t, :, 384:512] = W2i[128:256, 128:256]
        W3i = W3[i]
        slabt[t, :, 512:528] = W3i[0:128, :]
        slabt[t, :, 528:544] = W3i[128:256, :]
        biast[:, 4 * t + 0] = b2[i][0:128]
        biast[:, 4 * t + 1] = b2[i][128:256]
        biast[0:16, 4 * t + 2] = b3[i]
        b1p = b1[i] + _ROUND_INFO[i] @ W1[i][20:22]
        slabt[t, 0:16, 547:803] = W1[i][0:16]
        slabt[t, 16, 547:803] = b1p
        slabt[t, 17:21, 547:803] = W1[i][16:20]

    # ---- per-core (batch-sharded) tensors ----
    in_maps = []
    for c in range(NCORES):
        cols = slice(c * BC, (c + 1) * BC)
        mb = message_bytes[cols]          # (BC, 64)
        st = initial_state[cols]          # (BC, 16)
        st8 = np.zeros((128, GPW), dtype=np.float32)
        w0 = np.zeros((128, GPW), dtype=np.float32)
        msg8 = np.zeros((128, 16, GPW), dtype=np.float32)
        g0 = int(_SCHED[0])
        for g in range(2):
            gc = slice(g * GPW, (g + 1) * GPW)
            st8[64 * g:64 * g + 16, :] = st[gc].T
            w0[64 * g + 0, :] = 1.0
            for k in range(4):
                w0[64 * g + 1 + k, :] = mb[gc, 4 * g0 + k]
        # word strips: strip s = 2j+c rows hold chunk (p, c) at cols p*512
        for s in range(4):
            c2 = s % 2
            for p in range(2):
                ch = slice((2 * p + c2) * 512, (2 * p + c2 + 1) * 512)
                pc = slice(p * 512, (p + 1) * 512)
                msg8[32 * s + 0, :, pc] = 1.0
                for k in range(4):
                    msg8[32 * s + 1 + k, :, pc] = mb[ch, k::4].T  # word w
        msgt = np.empty((65, BC), dtype=np.float32)
        msgt[0:64] = mb.T
        msgt[64] = 1.0
        in_maps.append({
            "st8": st8.astype(F8NP),
            "w0": w0.astype(F8NP),
            "msg8": msg8.astype(F8NP),
            "wsl8": wsl8,
            "bias8": bias8,
            "msgt": msgt,
            "slabt": slabt,
            "biast": biast,
        })
    return in_maps


def kernel(message_bytes, initial_state, W1, b1, W2, b2, W3, b3):
    from concourse.bass_utils import run_bass_kernel_spmd

    if "nc" not in _COMPILED:
        _COMPILED["nc"] = _build()
    nc = _COMPILED["nc"]

    in_maps = _prep_host(
        message_bytes, initial_state, W1, b1, W2, b2, W3, b3)
    res = run_bass_kernel_spmd(nc, in_maps, list(range(NCORES)))
    out = np.concatenate([res.results[c]["out"] for c in range(NCORES)], axis=1)
    return np.ascontiguousarray(out.T)  # (B, 16) float32
